# revision 24
# baseline (speedup 1.0000x reference)
"""LinearAttention Trainium2 kernel (8 NeuronCores, data-parallel over batch).

Math (per batch b of 16; reference reshapes [b,c,64,64] -> [b,c,n], n=4096):
  qkv = w_qkv @ x_b                       # [384, n]
  q, k, v = rows [0:128], [128:256], [256:384]   (4 heads x 32 dims)
  k = softmax(k, axis=n)  (per row)
  ctx[d,e]  = sum_n k[d,n] v[e,n]         (per head: block-diag 32x32 blocks)
  out[e,n]  = sum_d ctx[d,e] q[d,n]       (block-diag masked)
  y = w_out @ out + b_out                 # [256, n]

On-chip strategy per core (2 batches/core):
  - x arrives as int8 + per-512-column f32 scales (wire codec: quarters the
    axon-tunnel upload; absmax/127 with round-to-nearest); the 8 scales per
    row ride in the trailing 32 bytes of the row, bitcast into the int8
    tensor, so each direction is a single dram tensor (fewer axon RPCs).
    Each c-chunk is staged int8 then dequantized to f32r in SBUF by the
    scalar engine, after which all math is identical to the f32r pipeline.
  - kT|vT computed directly in transposed layout ([n-chunk=128, 256]) via
    matmul(lhsT=x_chunk, rhs=w_kvT), f32r (1 cyc/row at free>=256).
  - softmax without max-subtraction (values are N(0,1)-scale; exp is safe):
    exp on ACT while copying psum->sbuf; 1/Z folded into ctx rows later.
  - Z comes free: vt tiles carry a ones-column at stride 129; the ctx
    accumulation's 256-wide rhs window picks it up as output column 128.
  - ctx masked to block-diagonal + scaled by 1/Z into a [128,128] lhsT.
  - bias folded into the final matmul via a K=1 pre-matmul (b x ones-row).
  - y leaves the chip int8 per-512-column-tile + f32 scales (absmax/127;
    the f32->int8 convert rounds-to-nearest-even with saturation), cutting
    the download 4x; the host dequantizes.

Runtime path: the axon redirect of run_bass_kernel_spmd goes through
bass2jax.run_bass_via_pjrt, which per call re-jits a fresh closure
(~1s of retrace + neuron recompile), uploads host-built zero buffers for
output donation (64MB), and refetches outputs redundantly. kernel.py
installs a drop-in replacement that caches the jitted executable per
Bass module, materializes the donated zero outputs on device, caches
unchanged small inputs (weights) as committed device arrays, and fetches
output shards with a thread pool. Everything the module computes still
runs on the NeuronCores via run_bass_kernel_spmd.
"""
import os
import sys

for _p in ("/opt/trn_rl_repo", "/root/.axon_site/_ro/trn_rl_repo"):
    if os.path.isdir(_p) and _p not in sys.path:
        sys.path.insert(0, _p)

from concurrent.futures import ThreadPoolExecutor

import numpy as np
import concourse.bass as bass
import concourse.bacc as bacc
import concourse.tile as tile
from concourse import mybir
from concourse import bass2jax
from concourse.bass_utils import run_bass_kernel_spmd

F32 = mybir.dt.float32
F32R = mybir.dt.float32r
I8 = mybir.dt.int8
EXP = mybir.ActivationFunctionType.Exp

NCORES = 8
B = 16
BPC = B // NCORES  # batches per core
C = 256
HID = 128
N = 4096
NT = 512
NCH = N // 128  # 32 n-chunks
NTC = N // NT  # 8 output column tiles (one scale each)

_NC_CACHE = {}


def build_nc():
    nc = bacc.Bacc()
    # x and yq rows carry their 8 f32 wire-codec scales bitcast into the
    # trailing 32 bytes (one tensor per direction -> fewer axon RPCs)
    x = nc.declare_dram_parameter("x", [BPC, C, N + 32], I8, isOutput=False)
    wq = nc.declare_dram_parameter("wqkvT", [C, 3 * HID], F32R, isOutput=False)
    wo = nc.declare_dram_parameter("woutT", [HID, C], F32R, isOutput=False)
    bb = nc.declare_dram_parameter("bias", [1, C], F32R, isOutput=False)
    idn = nc.declare_dram_parameter("ident", [HID, HID], F32R, isOutput=False)
    yq = nc.declare_dram_parameter("yq", [BPC, C, N + 32], I8, isOutput=True)

    with tile.TileContext(nc) as tc:
        with (
            tc.tile_pool(name="singles", bufs=1) as singles,
            tc.tile_pool(name="xst", bufs=2) as xst,
            tc.tile_pool(name="xp", bufs=2) as xp,
            tc.tile_pool(name="big", bufs=2) as big,
            tc.tile_pool(name="small", bufs=2) as small,
            tc.tile_pool(name="fin", bufs=4) as fin,
            tc.tile_pool(name="qs", bufs=4) as qs,
            tc.tile_pool(name="ps_kv", bufs=3, space="PSUM") as ps_kv,
            tc.tile_pool(name="ps_pq", bufs=2, space="PSUM") as ps_pq,
            tc.tile_pool(name="ps_ctx", bufs=1, space="PSUM") as ps_ctx,
            tc.tile_pool(name="ps_f", bufs=2, space="PSUM") as ps_f,
        ):
            w_sb = singles.tile([128, 2, 384], F32R)
            nc.sync.dma_start(out=w_sb, in_=wq[:].rearrange("(j p) o -> p j o", p=128))
            wo_sb = singles.tile([128, 256], F32R)
            nc.sync.dma_start(out=wo_sb, in_=wo[:])
            b_sb = singles.tile([1, 256], F32R)
            nc.sync.dma_start(out=b_sb, in_=bb[:])
            id_sb = singles.tile([HID, HID], F32R)
            nc.sync.dma_start(out=id_sb, in_=idn[:])
            # memset can't produce f32r; seed constants via f32 then copy
            scratch = singles.tile([128, 512], F32)
            nc.vector.memset(scratch, 1.0)
            ones_sb = singles.tile([1, 512], F32R)
            nc.vector.tensor_copy(out=ones_sb, in_=scratch[0:1, :])
            ones32 = singles.tile([128, 32], F32R)
            nc.vector.tensor_copy(out=ones32, in_=scratch[:, 0:32])
            nc.vector.memset(scratch, 0.0)
            zeros128 = singles.tile([128, 128], F32R)
            nc.vector.tensor_copy(out=zeros128, in_=scratch[:, 0:128])

            state = {}

            def ph_load(b):
                x_sb = xp.tile([128, 2, N], F32R, tag="x", name=f"x{b}")
                for j in range(2):
                    xh = xst.tile([128, N], I8, tag="xh", name=f"xh{b}_{j}")
                    sx8 = xst.tile([128, 32], I8, tag="sx", name=f"sx{b}_{j}")
                    nc.sync.dma_start(
                        out=sx8, in_=x[b, 128 * j : 128 * (j + 1), N : N + 32]
                    )
                    sx = sx8[:, :].bitcast(F32)
                    for t in range(8):
                        w = N // 8
                        nc.sync.dma_start(
                            out=xh[:, t * w : (t + 1) * w],
                            in_=x[b, 128 * j : 128 * (j + 1), t * w : (t + 1) * w],
                        )
                    for t in range(NTC):
                        nc.scalar.activation(
                            out=x_sb[:, j, t * NT : (t + 1) * NT],
                            in_=xh[:, t * NT : (t + 1) * NT],
                            func=mybir.ActivationFunctionType.Copy,
                            scale=sx[:, t : t + 1],
                        )
                state[b] = {"x": x_sb}

            def ph_kv(b):
                st = state[b]
                x_sb = st["x"]
                ktE = big.tile([128, N], F32R, tag="ktE", name=f"ktE{b}")
                vt = big.tile([128, NCH * 129 + 127], F32R, tag="vt", name=f"vt{b}")
                vt129 = vt[:, 0 : NCH * 129].rearrange("p (c s) -> p c s", s=129)
                nc.vector.tensor_copy(out=vt129[:, :, 128:129], in_=ones32.unsqueeze(2))
                nc.vector.tensor_copy(out=vt[:, NCH * 129 :], in_=zeros128[:, 0:127])
                for s in range(16):
                    kv_ps = ps_kv.tile([128, 2, 256], F32, tag="kv", name=f"kv{b}_{s}")
                    for i2 in range(2):
                        i = 2 * s + i2
                        for j in range(2):
                            nc.tensor.matmul(
                                kv_ps[:, i2, :],
                                x_sb[:, j, i * 128 : (i + 1) * 128],
                                w_sb[:, j, 128:384],
                                start=(j == 0),
                                stop=(j == 1),
                            )
                    nc.scalar.activation(
                        out=ktE[:, 2 * s * 128 : (2 * s + 2) * 128].rearrange(
                            "p (c d) -> p c d", d=128
                        ),
                        in_=kv_ps[:, :, 0:128],
                        func=EXP,
                    )
                    nc.vector.tensor_copy(
                        out=vt129[:, 2 * s : 2 * s + 2, 0:128],
                        in_=kv_ps[:, :, 128:256],
                    )
                st["ktE"], st["vt"] = ktE, vt

            def ph_q(b):
                st = state[b]
                x_sb = st["x"]
                q_sb = big.tile([128, N], F32R, tag="q", name=f"q{b}")
                for t in range(8):
                    q_ps = ps_pq.tile([128, NT], F32, tag="pq", name=f"qp{b}_{t}")
                    for j in range(2):
                        nc.tensor.matmul(
                            q_ps,
                            w_sb[:, j, 0:128],
                            x_sb[:, j, t * NT : (t + 1) * NT],
                            start=(j == 0),
                            stop=(j == 1),
                        )
                    nc.scalar.copy(out=q_sb[:, t * NT : (t + 1) * NT], in_=q_ps)
                st["q"] = q_sb

            def ph_ctx(b):
                st = state[b]
                ktE, vt = st["ktE"], st["vt"]
                ctx_ps = ps_ctx.tile([128, 256], F32, tag="ctx", name=f"ctx{b}")
                for i in range(NCH):
                    nc.tensor.matmul(
                        ctx_ps,
                        ktE[:, i * 128 : (i + 1) * 128],
                        vt[:, i * 129 : i * 129 + 256],
                        start=(i == 0),
                        stop=(i == NCH - 1),
                    )
                rz = small.tile([128, 1], F32, tag="rz", name=f"rz{b}")
                nc.vector.reciprocal(out=rz, in_=ctx_ps[:, 128:129])
                ctxm = small.tile([128, 128], F32R, tag="ctxm", name=f"ctxm{b}")
                nc.vector.tensor_copy(out=ctxm, in_=zeros128)
                for h in range(4):
                    sl = slice(32 * h, 32 * h + 32)
                    nc.vector.tensor_scalar_mul(
                        out=ctxm[sl, sl], in0=ctx_ps[sl, sl], scalar1=rz[sl, :]
                    )
                ctxt_ps = ps_pq.tile([128, 128], F32R, tag="pq", name=f"ct{b}")
                nc.tensor.transpose(ctxt_ps, ctxm, id_sb)
                ctxmT = small.tile([128, 128], F32R, tag="ctxmT", name=f"cT{b}")
                nc.vector.tensor_copy(out=ctxmT, in_=ctxt_ps)
                wt_ps = ps_pq.tile([128, 256], F32, tag="pq", name=f"wtp{b}")
                nc.tensor.matmul(wt_ps, ctxmT, wo_sb, start=True, stop=True)
                wt_sb = small.tile([128, 256], F32R, tag="wt", name=f"wt{b}")
                nc.vector.tensor_copy(out=wt_sb, in_=wt_ps)
                st["wt"] = wt_sb

            def ph_fin(b):
                st = state[b]
                q_sb, wt_sb = st["q"], st["wt"]
                for t in range(8):
                    for o in range(2):
                        f_ps = ps_f.tile([128, NT], F32, tag="f", name=f"f{b}_{t}_{o}")
                        nc.tensor.matmul(
                            f_ps,
                            b_sb[:, o * 128 : (o + 1) * 128],
                            ones_sb,
                            start=True,
                            stop=False,
                        )
                        nc.tensor.matmul(
                            f_ps,
                            wt_sb[:, o * 128 : (o + 1) * 128],
                            q_sb[:, t * NT : (t + 1) * NT],
                            start=False,
                            stop=True,
                        )
                        # int8 wire codec: per-partition absmax -> scale
                        m = qs.tile([128, 1], F32, tag="m", name=f"m{b}_{t}_{o}")
                        nc.vector.tensor_reduce(
                            out=m,
                            in_=f_ps,
                            axis=mybir.AxisListType.X,
                            op=mybir.AluOpType.max,
                            apply_absolute_value=True,
                        )
                        mc = qs.tile([128, 1], F32, tag="mc", name=f"mc{b}_{t}_{o}")
                        nc.vector.tensor_scalar_max(out=mc, in0=m, scalar1=1e-30)
                        r = qs.tile([128, 1], F32, tag="r", name=f"r{b}_{t}_{o}")
                        nc.vector.reciprocal(out=r, in_=mc)
                        s_sb = qs.tile([128, 1], F32, tag="s", name=f"s{b}_{t}_{o}")
                        nc.vector.tensor_scalar_mul(
                            out=s_sb, in0=mc, scalar1=1.0 / 127.0
                        )
                        q8 = fin.tile([128, NT], I8, tag="q8", name=f"q8{b}_{t}_{o}")
                        nc.vector.tensor_scalar(
                            out=q8,
                            in0=f_ps,
                            scalar1=r,
                            scalar2=127.0,
                            op0=mybir.AluOpType.mult,
                            op1=mybir.AluOpType.mult,
                        )
                        nc.sync.dma_start(
                            out=yq[b, o * 128 : (o + 1) * 128, t * NT : (t + 1) * NT],
                            in_=q8,
                        )
                        nc.sync.dma_start(
                            out=yq[
                                b,
                                o * 128 : (o + 1) * 128,
                                N + 4 * t : N + 4 * (t + 1),
                            ],
                            in_=s_sb[:, :].bitcast(I8),
                        )

            # software-pipelined emission across the two batches
            ph_load(0)
            ph_kv(0)
            ph_q(0)
            ph_load(1)
            ph_ctx(0)
            ph_kv(1)
            ph_fin(0)
            ph_q(1)
            ph_ctx(1)
            ph_fin(1)
    nc.compile()
    return nc


def get_nc():
    if "nc" not in _NC_CACHE:
        _NC_CACHE["nc"] = build_nc()
    return _NC_CACHE["nc"]


# ---------------------------------------------------------------------------
# Fast axon runner: drop-in replacement for bass2jax.run_bass_via_pjrt.
# Same lowering and execution path (_bass_exec_p custom call on
# jax.devices()), but the jitted executable, mesh, and zero-output maker
# are built once per Bass module instead of per call; donated zero output
# buffers are created on device (no 16-64MB upload per call); small
# unchanged inputs (weights) are kept as committed device arrays; output
# shards are fetched with a thread pool.
# ---------------------------------------------------------------------------

_RUNNERS: dict = {}
_ORIG_RUN_VIA_PJRT = bass2jax.run_bass_via_pjrt
_FETCH_POOL = ThreadPoolExecutor(max_workers=16)
_DEVCACHE_MAX_BYTES = 8 << 20  # cache inputs up to 8MB (weights, not x)
# When set, the pipeline runner calls _OUTPUT_HOOK(core, out_name, np_piece)
# from its fetch workers as each core's output lands — lets the caller
# dequantize per-core slices while other cores are still transferring.
_OUTPUT_HOOK = None


class _PipelineRunner:
    """Per-device pipeline: each core's upload -> exec -> download runs as
    an independent chain, so core i's download overlaps core j's upload
    (the axon tunnel is full-duplex at moderate concurrency) and the
    single collective dispatch barrier of shard_map is avoided."""

    def __init__(self, nc, n_cores):
        import jax
        import jax.numpy as jnp

        bass2jax.install_neuronx_cc_hook()
        assert nc.dbg_addr is None and nc.partition_id_tensor is None
        self.nc = nc
        self.n_cores = n_cores
        self.devices = list(jax.devices()[:n_cores])
        assert len(self.devices) == n_cores

        in_names: list = []
        in_shapes: list = []
        out_names: list = []
        out_avals: list = []
        out_shapes: list = []
        for alloc in nc.m.functions[0].allocations:
            if not isinstance(alloc, mybir.MemoryLocationSet):
                continue
            name = alloc.memorylocations[0].name
            shape = tuple(alloc.tensor_shape)
            dtype = mybir.dt.np(alloc.dtype)
            if alloc.kind == "ExternalInput":
                in_names.append(name)
                in_shapes.append((shape, dtype))
            elif alloc.kind == "ExternalOutput":
                out_names.append(name)
                out_avals.append(jax.core.ShapedArray(shape, dtype))
                out_shapes.append((shape, dtype))
        self.in_names = in_names
        self.out_names = out_names
        n_params = len(in_names)
        n_outs = len(out_names)
        all_in_names = list(in_names) + list(out_names)

        def _body(*args):
            outs = bass2jax._bass_exec_p.bind(
                *args,
                out_avals=tuple(out_avals),
                in_names=tuple(all_in_names),
                out_names=tuple(out_names),
                lowering_input_output_aliases=(),
                sim_require_finite=True,
                sim_require_nnan=True,
                nc=nc,
            )
            return tuple(outs)

        donate = tuple(range(n_params, n_params + n_outs))
        self.jbody = jax.jit(_body, donate_argnums=donate, keep_unused=True)
        # tiny per-device anchors: a jit with a device-committed operand runs
        # on that device, so these let us create zeros/warmup inputs on
        # device with no wire traffic and no out_shardings specialization
        self.anchors = [
            jax.device_put(np.zeros((1,), np.float32), d) for d in self.devices
        ]
        self.make_zeros = jax.jit(
            lambda ref: tuple(jnp.zeros(s, d) for s, d in out_shapes),
            keep_unused=True,
        )
        self.make_in_zeros = jax.jit(
            lambda ref: tuple(jnp.zeros(s, d) for s, d in in_shapes),
            keep_unused=True,
        )
        # warm up: compile + run once per device (zero wire traffic)
        for i in range(n_cores):
            dummy_ins = self.make_in_zeros(self.anchors[i])
            dummy_zeros = self.make_zeros(self.anchors[i])
            outs = self.jbody(*dummy_ins, *dummy_zeros)
            for o in outs:
                o.block_until_ready()
        self.per_dev_zeros = [self.make_zeros(a) for a in self.anchors]
        self.devcache: dict = {}

    def _core_input(self, name, core, value):
        import jax

        if isinstance(value, jax.Array):
            return value
        arr = np.asarray(value)
        if arr.nbytes <= _DEVCACHE_MAX_BYTES:
            key = (name, core)
            cached = self.devcache.get(key)
            if cached is not None and np.array_equal(cached[0], arr):
                return cached[1]
            dev = jax.device_put(arr, self.devices[core])
            self.devcache[key] = (arr.copy(), dev)
            return dev
        return jax.device_put(arr, self.devices[core])

    def __call__(self, in_maps):
        assert len(in_maps) == self.n_cores
        zeros_now = self.per_dev_zeros
        outs = []
        for i in range(self.n_cores):
            ins = [self._core_input(n, i, in_maps[i][n]) for n in self.in_names]
            outs.append(self.jbody(*ins, *zeros_now[i]))
        hook = _OUTPUT_HOOK

        def fetch(i):
            pieces = {}
            for j, name in enumerate(self.out_names):
                piece = np.asarray(outs[i][j])
                if hook is not None:
                    hook(i, name, piece)
                pieces[name] = piece
            return pieces

        futs = [_FETCH_POOL.submit(fetch, i) for i in range(self.n_cores)]
        results = [f.result() for f in futs]
        # refresh donated zero buffers for the next call (device-side memset,
        # overlaps the caller's host-side post/pre-processing)
        self.per_dev_zeros = [self.make_zeros(a) for a in self.anchors]
        return results


class _PjrtRunner:
    def __init__(self, nc, n_cores):
        import jax
        import jax.numpy as jnp
        from jax.sharding import Mesh, NamedSharding, PartitionSpec
        from jax.experimental.shard_map import shard_map

        bass2jax.install_neuronx_cc_hook()
        assert nc.dbg_addr is None or not nc.dbg_callbacks
        self.nc = nc
        self.n_cores = n_cores
        partition_name = (
            nc.partition_id_tensor.name if nc.partition_id_tensor else None
        )

        in_names: list = []
        out_names: list = []
        out_avals: list = []
        out_shapes: list = []
        for alloc in nc.m.functions[0].allocations:
            if not isinstance(alloc, mybir.MemoryLocationSet):
                continue
            name = alloc.memorylocations[0].name
            if alloc.kind == "ExternalInput":
                if name != partition_name and name != (
                    nc.dbg_addr.name if nc.dbg_addr is not None else None
                ):
                    in_names.append(name)
            elif alloc.kind == "ExternalOutput":
                shape = tuple(alloc.tensor_shape)
                dtype = mybir.dt.np(alloc.dtype)
                out_names.append(name)
                out_avals.append(jax.core.ShapedArray(shape, dtype))
                out_shapes.append((shape, dtype))
        self.in_names = list(in_names)
        self.out_names = out_names
        self.out_avals = out_avals
        n_params = len(in_names)
        n_outs = len(out_names)

        all_in_names = list(in_names) + list(out_names)
        if nc.dbg_addr is not None:
            all_in_names.append(nc.dbg_addr.name)
        if partition_name is not None:
            all_in_names.append(partition_name)

        def _body(*args):
            operands = list(args)
            if nc.dbg_addr is not None:
                operands.append(jnp.zeros((1, 2), jnp.uint32))
            if partition_name is not None:
                operands.append(bass2jax.partition_id_tensor())
            outs = bass2jax._bass_exec_p.bind(
                *operands,
                out_avals=tuple(out_avals),
                in_names=tuple(all_in_names),
                out_names=tuple(out_names),
                lowering_input_output_aliases=(),
                sim_require_finite=True,
                sim_require_nnan=True,
                nc=nc,
            )
            return tuple(outs)

        devices = jax.devices()[:n_cores]
        assert len(devices) == n_cores
        self.mesh = Mesh(np.asarray(devices), ("core",))
        self.sharding = NamedSharding(self.mesh, PartitionSpec("core"))
        in_specs = (PartitionSpec("core"),) * (n_params + n_outs)
        out_specs = (PartitionSpec("core"),) * n_outs
        donate = tuple(range(n_params, n_params + n_outs))
        self.sharded = jax.jit(
            shard_map(
                _body,
                mesh=self.mesh,
                in_specs=in_specs,
                out_specs=out_specs,
                check_rep=False,
            ),
            donate_argnums=donate,
            keep_unused=True,
        )

        zero_sharding = tuple(self.sharding for _ in range(n_outs))
        self.make_zeros = jax.jit(
            lambda: tuple(
                jnp.zeros((n_cores * s[0], *s[1:]), d) for s, d in out_shapes
            ),
            out_shardings=zero_sharding if n_outs else None,
        )
        self.devcache: dict = {}
        self._zeros_next = None

    def _global_input(self, name, in_maps):
        import jax

        parts = [m[name] for m in in_maps]
        if all(isinstance(p, jax.Array) for p in parts):
            # pre-put single-device pieces (e.g. uploaded concurrently with
            # host-side quantization) — assemble zero-copy
            shape = (self.n_cores * parts[0].shape[0], *parts[0].shape[1:])
            return jax.make_array_from_single_device_arrays(
                shape, self.sharding, list(parts)
            )
        parts = [np.asarray(p) for p in parts]
        glob = np.concatenate(parts, axis=0)
        if glob.nbytes <= _DEVCACHE_MAX_BYTES:
            cached = self.devcache.get(name)
            if cached is not None and np.array_equal(cached[0], glob):
                return cached[1]
            dev = jax.device_put(glob, self.sharding)
            self.devcache[name] = (glob, dev)
            return dev
        return glob

    def __call__(self, in_maps):
        assert len(in_maps) == self.n_cores
        ins = [self._global_input(name, in_maps) for name in self.in_names]
        zeros = self._zeros_next or self.make_zeros()
        out_arrs = self.sharded(*ins, *zeros)
        fetched = [np.empty(a.shape, a.dtype) for a in out_arrs]
        futs = [
            _FETCH_POOL.submit(
                lambda buf, sh: buf.__setitem__(sh.index, np.asarray(sh.data)),
                fetched[i],
                sh,
            )
            for i in range(len(out_arrs))
            for sh in out_arrs[i].addressable_shards
        ]
        # prefetch next call's donated zero buffers; the device-side memset
        # overlaps this call's wire transfers
        self._zeros_next = self.make_zeros()
        for f in futs:
            f.result()
        per_core = []
        for c in range(self.n_cores):
            per_core.append(
                {
                    name: fetched[i].reshape(
                        self.n_cores, *self.out_avals[i].shape
                    )[c]
                    for i, name in enumerate(self.out_names)
                }
            )
        return per_core


def _fast_run_bass_via_pjrt(nc, in_maps, n_cores):
    key = (id(nc), n_cores)
    runner = _RUNNERS.get(key)
    if runner is None:
        try:
            runner = _PipelineRunner(nc, n_cores)
        except Exception:
            try:
                runner = _PjrtRunner(nc, n_cores)
            except Exception:
                return _ORIG_RUN_VIA_PJRT(nc, in_maps, n_cores)
        _RUNNERS[key] = runner
    return runner(in_maps)


bass2jax.run_bass_via_pjrt = _fast_run_bass_via_pjrt


_HOST_BUFS = {}


def _hbuf(name, shape, dtype):
    buf = _HOST_BUFS.get(name)
    if buf is None or buf.shape != shape or buf.dtype != dtype:
        buf = np.empty(shape, dtype)
        _HOST_BUFS[name] = buf
    return buf


def make_in_maps(x, w_qkv, w_out, b_out):
    # int8 wire codec for x: per-512-column absmax scales, RNE rounding.
    # Quantize per core and start that core's upload immediately so the
    # host-side quantization overlaps the wire transfer.
    import jax

    devices = jax.devices()[:NCORES]
    x4 = np.asarray(x, np.float32).reshape(B, C, NTC, NT)
    tmp = _hbuf("qtmp", (BPC, C, NTC, NT), np.float32)
    xq = _hbuf("xq", (B, C, N + 32), np.int8)
    x_futs = []
    for i in range(NCORES):
        bsl = slice(i * BPC, (i + 1) * BPC)
        m = x4[bsl].max(-1, keepdims=True)
        np.maximum(m, -x4[bsl].min(-1, keepdims=True), out=m)
        np.maximum(m, 1e-30, out=m)
        np.multiply(x4[bsl], 127.0 / m, out=tmp)
        np.rint(tmp, out=tmp)
        xq[bsl, :, :N] = tmp.reshape(BPC, C, N)
        xq[bsl, :, N:].view(np.float32)[...] = m[..., 0]
        xq[bsl, :, N:].view(np.float32)[...] *= 1.0 / 127.0
        x_futs.append(
            _FETCH_POOL.submit(jax.device_put, xq[bsl], devices[i])
        )
    wqkvT = np.ascontiguousarray(np.asarray(w_qkv, np.float32).T)
    woutT = np.ascontiguousarray(np.asarray(w_out, np.float32).T)
    bias = np.ascontiguousarray(np.asarray(b_out, np.float32).reshape(1, C))
    ident = np.eye(HID, dtype=np.float32)
    return [
        {
            "x": x_futs[i].result(),
            "wqkvT": wqkvT,
            "woutT": woutT,
            "bias": bias,
            "ident": ident,
        }
        for i in range(NCORES)
    ]


def postprocess(res):
    packed = np.concatenate([res.results[i]["yq"] for i in range(NCORES)], axis=0)
    q4 = np.lib.stride_tricks.as_strided(
        packed[:, :, :N],
        shape=(B, C, NTC, NT),
        strides=(C * (N + 32), N + 32, NT, 1),
    )
    s = packed[:, :, N:].view(np.float32)
    y = np.multiply(q4, s[..., None])
    return y.reshape(B, C, 64, 64)


def kernel(x, w_qkv, w_out, b_out):
    global _OUTPUT_HOOK
    nc = get_nc()
    in_maps = make_in_maps(x, w_qkv, w_out, b_out)
    y_out = np.empty((B, C, NTC, NT), np.float32)
    done = [False] * NCORES

    def hook(core, name, piece):
        if name != "yq":
            return
        q4 = np.lib.stride_tricks.as_strided(
            piece[:, :, :N],
            shape=(BPC, C, NTC, NT),
            strides=(C * (N + 32), N + 32, NT, 1),
        )
        s = piece[:, :, N:].view(np.float32)
        np.multiply(q4, s[..., None], out=y_out[core * BPC : (core + 1) * BPC])
        done[core] = True

    _OUTPUT_HOOK = hook
    try:
        res = run_bass_kernel_spmd(nc, in_maps, list(range(NCORES)))
    finally:
        _OUTPUT_HOOK = None
    if not all(done):  # non-pipeline fallback path was used
        return postprocess(res)
    return y_out.reshape(B, C, 64, 64)
